# revision 36
# baseline (speedup 1.0000x reference)
"""Trainium2 Bass kernel for nn_MultiHeadAttention (B=4, S=2048, D=1024, H=16).

Sharding: 8 cores = batch(4) x head-half(2).  Each core computes, for its
batch element, 8 of the 16 heads: QKV projections against column-sliced
weights, causal attention, and the output projection against the matching
row-slice of Wo.  The two partial outputs per batch element are summed on
the host (replaces the tensor-parallel all-reduce), and Wo_b is added there.

Attention runs in the transposed-scores layout scoresT[k, q]; the softmax
denominator comes free from an all-ones column appended to V (row 64 of the
PV psum accumulator).  Heads are processed in PAIRS sharing one [128, 1024]
scores psum tile (head A in cols 0:512, head B in 512:1024) so one ACTIVATE
exps both heads' scores; q is chunked at 512.

The whole kernel is software-pipelined around the ACT engine's exp stream
(the irreducible ~120us of work): Q/K projections for the NEXT head pair
and output-projection tiles for finished pairs are injected as fill between
attention steps so the PE never idles (idle gaps also drop the PE's DVFS
p-state from 2.4 to 1.2 GHz).  PSUM: 2 scores bufs (4 banks) + 1 shared PV
accumulator (2 banks) + 2 fill bufs (2 banks).
"""

import sys

if "/opt/trn_rl_repo" not in sys.path:
    sys.path.insert(0, "/opt/trn_rl_repo")

import numpy as np
import ml_dtypes

B, S, D = 4, 2048, 1024
H, HD = 16, 64
HH = H // 2          # heads per core
DH = D // 2          # local attention feature dim (HH * HD)
N_CORES = 8
QC = 512             # q-chunk per attention pass (1 psum bank per head)

# matmul dtype mode: "bf16" (fast, ~3e-3 rel err) | "f32" (exact, 4x PE cost)
DT_MODE = "bf16"

_CACHE = {}


def _build(dt_mode):
    import concourse.bass as bass
    import concourse.mybir as mybir
    from concourse import bacc
    from concourse.tile import TileContext
    from concourse.masks import make_upper_triangular

    F32 = mybir.dt.float32
    if dt_mode == "bf16":
        DT = mybir.dt.bfloat16
    elif dt_mode == "f32":
        DT = mybir.dt.float32
    else:
        raise ValueError(dt_mode)
    FP8 = mybir.dt.float8e4
    DR = mybir.MatmulPerfMode.DoubleRow

    ADD = mybir.AluOpType.add
    MULT = mybir.AluOpType.mult
    EXP = mybir.ActivationFunctionType.Exp

    nc = bacc.Bacc("TRN2", target_bir_lowering=False, debug=False,
                   num_devices=N_CORES)

    xT = nc.dram_tensor("xT", [D, S], DT, kind="ExternalInput").ap()
    wq = nc.dram_tensor("wq", [D, DH], DT, kind="ExternalInput").ap()
    wk = nc.dram_tensor("wk", [D, DH], DT, kind="ExternalInput").ap()
    wv = nc.dram_tensor("wv", [D, DH], DT, kind="ExternalInput").ap()
    wo = nc.dram_tensor("wo", [DH, D], DT, kind="ExternalInput").ap()
    bq = nc.dram_tensor("bq", [128, DH // 128], F32, kind="ExternalInput").ap()
    bk = nc.dram_tensor("bk", [128, DH // 128], F32, kind="ExternalInput").ap()
    bv = nc.dram_tensor("bv", [128, DH], F32, kind="ExternalInput").ap()
    out = nc.dram_tensor("out", [S, D], F32, kind="ExternalOutput").ap()

    ND = D // 128        # 8 contraction tiles over D
    NS = S // 128        # 16 s-blocks
    NJ = DH // 128       # 4 head-pair tiles of the local 512 dim
    NSC = S // 512       # 4 columns of 512 over S
    NP = S // QC         # 4 q-chunk passes

    with TileContext(nc) as tc:
        with (
            tc.tile_pool(name="persist", bufs=1) as pp,
            tc.tile_pool(name="xt", bufs=ND * NSC) as pxt,
            tc.tile_pool(name="wqk", bufs=2 * ND) as pwqk,
            tc.tile_pool(name="wv", bufs=ND) as pwv,
            tc.tile_pool(name="wo", bufs=NJ) as pwo,
            tc.tile_pool(name="qT", bufs=NJ) as pqT,
            tc.tile_pool(name="kT", bufs=NJ) as pkT,
            tc.tile_pool(name="vaug", bufs=NS) as pv,
            tc.tile_pool(name="attnT", bufs=NJ) as pattnT,
            tc.tile_pool(name="exp", bufs=3) as pexp,
            tc.tile_pool(name="au", bufs=2) as pau,
            tc.tile_pool(name="dn", bufs=4) as pdn,
            tc.tile_pool(name="bc", bufs=2) as pbc,
            tc.tile_pool(name="ostage", bufs=4) as post,
            tc.tile_pool(name="scps", bufs=2, space="PSUM") as pscps,
            tc.tile_pool(name="atps", bufs=1, space="PSUM") as patps,
            tc.tile_pool(name="auxps", bufs=2, space="PSUM") as pauxps,
        ):
            # ---- input DMAs (ordered by first use) ----
            # x on the Sync queue; weights on the (otherwise idle) Scalar
            # and Vector queues so 35 dma_starts don't serialize at ~650ns
            # each on one sequencer.
            wq_t, wk_t = [], []
            for db in range(ND):
                tq = pwqk.tile([128, DH], DT, tag="wqk", name=f"wq{db}")
                nc.scalar.dma_start(tq[:], wq[db * 128:(db + 1) * 128, :])
                wq_t.append(tq)
            for db in range(ND):
                tk = pwqk.tile([128, DH], DT, tag="wqk", name=f"wk{db}")
                nc.scalar.dma_start(tk[:], wk[db * 128:(db + 1) * 128, :])
                wk_t.append(tk)
            bq_t = pp.tile([128, NJ], F32, tag="bq")
            nc.scalar.dma_start(bq_t[:], bq[:])
            bk_t = pp.tile([128, NJ], F32, tag="bk")
            nc.scalar.dma_start(bk_t[:], bk[:])
            bv_t = pp.tile([128, DH], F32, tag="bv")
            nc.scalar.dma_start(bv_t[:], bv[:])
            # x split into per-(db, sc) chunk tiles so the first Q/K
            # projection chunk waits on 1MB of x, not 4MB (each tile has a
            # single DMA writer -> readers gate only on their own chunk).
            xt_c = [[pxt.tile([128, 512], DT, tag="xt",
                              name=f"xt{db}_{sc}") for sc in range(NSC)]
                    for db in range(ND)]
            for db in range(ND):
                nc.sync.dma_start(xt_c[db][0][:],
                                  xT[db * 128:(db + 1) * 128, 0:512])
            wv_t = []
            for db in range(ND):
                t = pwv.tile([128, DH], DT, tag="wv", name=f"wv{db}")
                nc.sync.dma_start(t[:], wv[db * 128:(db + 1) * 128, :])
                wv_t.append(t)
            for sc in range(1, NSC):
                for db in range(ND):
                    nc.sync.dma_start(
                        xt_c[db][sc][:],
                        xT[db * 128:(db + 1) * 128, sc * 512:(sc + 1) * 512])
            wo_t = []
            for db in range(NJ):
                t = pwo.tile([128, D], DT, tag="wo", name=f"wo{db}")
                nc.sync.dma_start(t[:], wo[db * 128:(db + 1) * 128, :])
                wo_t.append(t)

            # ---- constants ----
            ones_t = pp.tile([128, HH], F32, tag="ones")
            nc.gpsimd.memset(ones_t[:], 1.0)
            ones1_t = pp.tile([1, 64], F32, tag="ones1")
            nc.gpsimd.memset(ones1_t[:], 1.0)
            # causal mask for diagonal 128x128 squares of scoresT[k, q]:
            # valid (k <= q) <=> partition p <= free f -> upper-tri incl
            # diag; two side-by-side copies (one per head of a pair).
            mask_f = pp.tile([128, 128], F32, tag="maskf")
            make_upper_triangular(nc, mask_f[:], val=1.0, diag=True)
            mask2 = pp.tile([128, 256], DT, tag="mask2")
            nc.vector.tensor_copy(mask2[:, 0:128], mask_f[:])
            nc.vector.tensor_copy(mask2[:, 128:256], mask_f[:])
            mask23 = mask2[:].rearrange("p (h c) -> p h c", h=2)

            # persistent activation buffers
            qT_t = [pqT.tile([128, S], DT, tag="qT", name=f"qT{i}")
                    for i in range(NJ)]
            kT_t = [pkT.tile([128, S], DT, tag="kT", name=f"kT{i}")
                    for i in range(NJ)]
            v_t = [pv.tile([128, HH * (HD + 1)], DT, tag="vaug",
                           name=f"vaug{i}") for i in range(NS)]
            aT_t = [pattnT.tile([128, S], DT, tag="attnT", name=f"attnT{i}")
                    for i in range(NJ)]

            # ---------- fill-work generators (2 matmuls per piece) ----------
            def qk_pieces(j):
                """Q/K projection for head pair j: chunks of ~0.4us pieces.
                A chunk = one psum accumulation group (must not be split
                around another aux-pool allocation)."""
                chunks = []
                for nm, w_t, bias_t, dstT in (
                    ("q", wq_t, bq_t, qT_t), ("k", wk_t, bk_t, kT_t)
                ):
                    for sc in range(NSC):
                        box = {}
                        pieces = []
                        for db0 in range(0, ND, 2):
                            def piece(db0=db0, nm=nm, w_t=w_t, bias_t=bias_t,
                                      dstT=dstT, sc=sc, j=j, box=box):
                                if db0 == 0:
                                    box["t"] = pauxps.tile(
                                        [128, 512], F32, tag="aux",
                                        name=f"qk{nm}{j}_{sc}")
                                for db in (db0, db0 + 1):
                                    nc.tensor.matmul(
                                        box["t"][:],
                                        lhsT=w_t[db][:, j * 128:(j + 1) * 128],
                                        rhs=xt_c[db][sc][:],
                                        start=(db == 0), stop=(db == ND - 1),
                                    )
                                if db0 == ND - 2:
                                    nc.vector.tensor_scalar_add(
                                        dstT[j][:, sc * 512:(sc + 1) * 512],
                                        box["t"][:], bias_t[:, j:j + 1],
                                    )
                            pieces.append(piece)
                        chunks.append(pieces)
                return chunks

            def v_pieces(sb):
                """V projection for s-block sb (one chunk of 4 pieces)."""
                pieces = []
                box = {}
                for db0 in range(0, ND, 2):
                    def piece(db0=db0, sb=sb, box=box):
                        if db0 == 0:
                            box["t"] = pauxps.tile([128, 512], F32, tag="aux",
                                                   name=f"vps{sb}")
                        for db in (db0, db0 + 1):
                            nc.tensor.matmul(
                                box["t"][:],
                                lhsT=xt_c[db][sb // 4][:, (sb % 4) * 128:(sb % 4 + 1) * 128],
                                rhs=wv_t[db][:],
                                start=(db == 0), stop=(db == ND - 1),
                            )
                        if db0 == ND - 2:
                            vt = v_t[sb]
                            v3 = vt[:].rearrange("p (h e) -> p h e", e=HD + 1)
                            nc.vector.tensor_tensor(
                                v3[:, :, 0:HD],
                                box["t"][:].rearrange("p (h e) -> p h e", e=HD),
                                bv_t[:].rearrange("p (h e) -> p h e", e=HD),
                                op=ADD,
                            )
                            nc.vector.tensor_copy(
                                v3[:, :, HD:HD + 1],
                                ones_t[:].rearrange("p (h e) -> p h e", e=1),
                            )
                    pieces.append(piece)
                return [pieces]

            def outproj_pieces(sb):
                """Output projection for s-block sb: 2 chunks of 2 pieces."""
                chunks = []
                for jc in range(D // 512):
                    box = {}
                    pieces = []
                    for db0 in range(0, NJ, 2):
                        def piece(db0=db0, sb=sb, jc=jc, box=box):
                            if db0 == 0:
                                box["t"] = pauxps.tile(
                                    [128, 512], F32, tag="aux",
                                    name=f"ops{sb}_{jc}")
                            for db in (db0, db0 + 1):
                                nc.tensor.matmul(
                                    box["t"][:],
                                    lhsT=aT_t[db][:, sb * 128:(sb + 1) * 128],
                                    rhs=wo_t[db][:, jc * 512:(jc + 1) * 512],
                                    start=(db == 0), stop=(db == NJ - 1),
                                )
                            if db0 == NJ - 2:
                                ot = post.tile([128, 512], F32, tag="ostage",
                                               name=f"ot{sb}_{jc}")
                                nc.vector.tensor_copy(ot[:], box["t"][:])
                                nc.sync.dma_start(
                                    out[sb * 128:(sb + 1) * 128,
                                        jc * 512:(jc + 1) * 512],
                                    ot[:],
                                )
                        pieces.append(piece)
                    chunks.append(pieces)
                return chunks

            # ---------- attention for one head pair, with fill ----------
            # pending_norm holds the deferred tail of the previous pass's
            # softmax-normalize (rank-1 PE broadcast of 1/den + gpsimd
            # multiplies).  Deferring it into the NEXT pass's instruction
            # stream keeps the in-order PE from blocking on the DVE
            # reciprocal, and using a PE matmul instead of gpsimd's
            # partition_broadcast keeps gpsimd on a single library
            # (UNLOAD_LIB/LOAD_LIB thrash costs ~13us per pass otherwise).
            pending_norm = []

            def attention_pair(j, fill, forced=None):
                """fill: per-pass CHUNK-lists (len NP); forced: optional
                {(p, kb): [chunks]} issued right after exp_pv(kb) of pass
                p (hard program-order deadlines for pass-0 operands)."""
                forced = forced or {}
                vcA = (2 * j) * (HD + 1)
                vcB = (2 * j + 1) * (HD + 1)
                for p in range(NP):
                    q0 = p * QC
                    nkb = (q0 + QC) // 128
                    at2 = patps.tile([65, 2 * QC], F32, tag="at",
                                     name=f"at{j}_{p}")
                    # flatten chunks; record the piece indices that are
                    # chunk boundaries (safe points for aux-psum reuse).
                    pfill = [pc for ch in fill[p] for pc in ch]
                    bounds = set()
                    n = 0
                    for ch in fill[p]:
                        bounds.add(n)
                        n += len(ch)
                    bounds.add(n)
                    fi = 0

                    def scores(kb):
                        k0 = kb * 128
                        lo = max(k0 - q0, 0)
                        sc2 = pscps.tile([128, 2 * QC], F32, tag="sc",
                                         name=f"sc{j}_{p}_{kb}")
                        for hi, hr in ((0, 0), (1, 64)):
                            nc.tensor.matmul(
                                sc2[:, hi * QC + lo:(hi + 1) * QC],
                                lhsT=kT_t[j][hr:hr + 64, k0:k0 + 128],
                                rhs=qT_t[j][hr:hr + 64, q0 + lo:q0 + QC],
                                start=True, stop=True,
                            )
                        return sc2

                    def exp_pv(kb, sc2):
                        k0 = kb * 128
                        lo = max(k0 - q0, 0)
                        et = pexp.tile([128, 2 * QC], DT, tag="exp",
                                       name=f"et{j}_{p}_{kb}")
                        et3 = et[:].rearrange("p (h c) -> p h c", h=2)
                        sc3 = sc2[:].rearrange("p (h c) -> p h c", h=2)
                        nc.scalar.activation(
                            et3[:, :, lo:QC], sc3[:, :, lo:QC],
                            EXP, scale=1.0 / np.sqrt(HD),
                        )
                        if k0 >= q0:
                            nc.vector.tensor_tensor(
                                et3[:, :, lo:lo + 128],
                                et3[:, :, lo:lo + 128],
                                mask23, op=MULT,
                            )
                        for hi, vc in ((0, vcA), (1, vcB)):
                            nc.tensor.matmul(
                                at2[0:65, hi * QC + lo:(hi + 1) * QC],
                                lhsT=v_t[kb][:, vc:vc + HD + 1],
                                rhs=et[:, hi * QC + lo:(hi + 1) * QC],
                                start=(kb == 0), stop=(kb == nkb - 1),
                            )

                    pend = {}
                    for kb in range(min(2, nkb)):
                        pend[kb] = scores(kb)
                    for kb in range(nkb):
                        want = min(len(pfill),
                                   ((kb + 1) * len(pfill))
                                   // max(nkb - 3, 2))
                        while fi < want:
                            pfill[fi]()
                            fi += 1
                        # exp_pv(kb) BEFORE scores(kb+2): the scps pool has
                        # 2 bufs, so scores(kb+2) reuses sc2(kb)'s buffer
                        # and its WAR dep must see exp(kb) already issued.
                        exp_pv(kb, pend.pop(kb))
                        fch = forced.get((p, kb))
                        if fch:
                            # close any open fill chunk first (aux psum
                            # groups must not interleave), then issue.
                            while fi not in bounds:
                                pfill[fi]()
                                fi += 1
                            for ch in fch:
                                for piece in ch:
                                    piece()
                        if kb == 2 and pending_norm:
                            # finish any open fill chunk first: the deferred
                            # normalize allocates from the same aux psum
                            # pool and must not land inside an open
                            # accumulation group.
                            while fi not in bounds:
                                pfill[fi]()
                                fi += 1
                            pending_norm.pop(0)()
                        if kb + 2 < nkb:
                            pend[kb + 2] = scores(kb + 2)
                    while fi < len(pfill):
                        pfill[fi]()
                        fi += 1

                    # one DVE copy frees the at2 psum slot; reciprocal of
                    # the denominator row runs now (DVE only); broadcast +
                    # multiplies are deferred one pass (see pending_norm).
                    au = pau.tile([65, 2 * QC], F32, tag="au",
                                  name=f"au{j}_{p}")
                    nc.vector.tensor_copy(au[:], at2[0:65, :])
                    dn = pdn.tile([1, 2 * QC], F32, tag="dn", name=f"dn{j}_{p}")
                    nc.vector.tensor_copy(dn[:], au[64:65, :])
                    rc = pdn.tile([1, 2 * QC], F32, tag="rc", name=f"rc{j}_{p}")
                    nc.vector.reciprocal_approx_fast(rc[:], dn[:])

                    def norm_tail(j=j, q0=q0, au=au, rc=rc, tn=f"{j}_{p}"):
                        bcb = pbc.tile([64, 2 * QC], F32, tag="bc",
                                       name=f"bc{tn}")
                        for c in range(2):
                            bp = pauxps.tile([64, QC], F32, tag="aux",
                                             name=f"bcps{tn}_{c}")
                            nc.tensor.matmul(
                                bp[:], lhsT=ones1_t[0:1, :],
                                rhs=rc[0:1, c * QC:(c + 1) * QC],
                                start=True, stop=True,
                            )
                            nc.vector.tensor_copy(
                                bcb[:, c * QC:(c + 1) * QC], bp[:])
                        for hi, hr in ((0, 0), (1, 64)):
                            nc.gpsimd.tensor_tensor(
                                aT_t[j][hr:hr + 64, q0:q0 + QC],
                                au[0:64, hi * QC:(hi + 1) * QC],
                                bcb[:, hi * QC:(hi + 1) * QC],
                                op=MULT,
                            )
                    pending_norm.append(norm_tail)

            # ---------------- schedule ----------------
            # prologue: only what pass 0 of window 0 needs up front --
            # q-chunk 0, k-chunk 0, V(0).  Everything else is forced into
            # window 0 just ahead of its first use, so the ACT exp stream
            # starts ~35us earlier.
            qk0 = qk_pieces(0)          # [q0..q3, k0..k3]
            for ch in (qk0[0], qk0[4], v_pieces(0)[0]):
                for piece in ch:
                    piece()

            vch = [v_pieces(sb)[0] for sb in range(1, NS)]   # V(1..15)
            qk1 = qk_pieces(1)
            forced = {
                (0, 0): [vch[0], qk0[1], qk0[5]],   # V1, q1, k1
                (0, 1): [vch[1]],                   # V2
                (0, 2): [vch[2]],                   # V3
                (1, 0): [qk0[2], qk0[6]],           # q2, k2
                (2, 0): [qk0[3], qk0[7]],           # q3, k3
            }
            fill = [vch[3:7] + qk1[0:2], vch[7:11] + qk1[2:4],
                    vch[11:15] + qk1[4:6], qk1[6:8]]
            attention_pair(0, fill, forced)

            # windows 1..2: attention(j) + QK(j+1) as fill.  Fill only the
            # first NP-1 passes so the next window's first scores never
            # wait on a bias-add landing at the window edge.
            for j in range(1, NJ - 1):
                qk = qk_pieces(j + 1)
                cuts = [0, 1, 3, 5, len(qk)]
                fill = [qk[cuts[p]:cuts[p + 1]] for p in range(NP)]
                attention_pair(j, fill)

            # window 3: attention(3) + out-proj of s-blocks, gated one FULL
            # pass after the pass that produced their aT columns so the
            # in-order PE never head-of-line blocks on the normalize chain
            # (pass p covers q-chunk p: sb 4(p-2)..4(p-1)-1 in pass p).
            fill = [[] for _ in range(NP)]
            for p in range(2, NP):
                for sb in range(4 * (p - 2), 4 * (p - 1)):
                    fill[p].extend(outproj_pieces(sb))
            attention_pair(NJ - 1, fill)

            # epilogue: out-proj for s-blocks 8..11 (their aT was written by
            # drained normalizes), then flush the final pass's normalize,
            # then the last s-blocks that depend on it.
            for sb in range(4 * (NP - 2), 4 * (NP - 1)):
                for ch in outproj_pieces(sb):
                    for piece in ch:
                        piece()
            while pending_norm:
                pending_norm.pop(0)()
            for sb in range(4 * (NP - 1), NS):
                for ch in outproj_pieces(sb):
                    for piece in ch:
                        piece()

    nc.compile()
    return nc


def _get_nc(dt_mode):
    if dt_mode not in _CACHE:
        _CACHE[dt_mode] = _build(dt_mode)
    return _CACHE[dt_mode]


def make_in_maps(x, Wq_w, Wq_b, Wk_w, Wk_b, Wv_w, Wv_b, Wo_w, Wo_b, np_dt):
    in_maps = []
    for core in range(N_CORES):
        b, half = core // 2, core % 2
        sl = slice(half * DH, (half + 1) * DH)
        in_maps.append({
            "xT": np.ascontiguousarray(x[b].T).astype(np_dt),
            "wq": np.ascontiguousarray(Wq_w[:, sl]).astype(np_dt),
            "wk": np.ascontiguousarray(Wk_w[:, sl]).astype(np_dt),
            "wv": np.ascontiguousarray(Wv_w[:, sl]).astype(np_dt),
            "wo": np.ascontiguousarray(Wo_w[sl, :]).astype(np_dt),
            "bq": np.ascontiguousarray(Wq_b[sl].reshape(-1, 128).T),
            "bk": np.ascontiguousarray(Wk_b[sl].reshape(-1, 128).T),
            "bv": np.broadcast_to(Wv_b[sl], (128, DH)).copy(),
        })
    return in_maps


def kernel(x, Wq_w, Wq_b, Wk_w, Wk_b, Wv_w, Wv_b, Wo_w, Wo_b):
    from concourse.bass_utils import run_bass_kernel_spmd

    np_dt = ml_dtypes.bfloat16 if DT_MODE == "bf16" else np.float32

    args = [np.asarray(a, np.float32) for a in
            (x, Wq_w, Wq_b, Wk_w, Wk_b, Wv_w, Wv_b, Wo_w, Wo_b)]
    x, Wq_w, Wq_b, Wk_w, Wk_b, Wv_w, Wv_b, Wo_w, Wo_b = args

    nc = _get_nc(DT_MODE)
    in_maps = make_in_maps(x, Wq_w, Wq_b, Wk_w, Wk_b, Wv_w, Wv_b, Wo_w, Wo_b,
                           np_dt)
    res = run_bass_kernel_spmd(nc, in_maps, list(range(N_CORES)))

    out = np.empty((B, S, D), np.float32)
    for b in range(B):
        out[b] = res.results[2 * b]["out"] + res.results[2 * b + 1]["out"] + Wo_b
    return out
